# revision 28
# baseline (speedup 1.0000x reference)
"""Trainium2 Bass kernel for the BSG word2gauss-style hinge/KL loss.

Strategy (data-parallel over 8 NeuronCores):
  - Host precomputes gather tables (batch-independent weight prep).
    Key algebra: 2*kl + L = A_b*iv_w + h'_b . g'_w + c_w - lsg_b with
      A_b  = exp(lsg_b) + sum(mu_b^2)
      g'_w = -2*iv_w*(wf[:, :L] @ tm_w)  in R^{H+1}   (projected type mean)
      c_w  = sq_w*iv_w + lv_w
    so the per-(row, word) interaction is a 51-dim dot with h' = [h; 1]
    instead of a 100-dim dot with mu, and no mu2 scaling pass is needed.
    Tables, 128B rows (fp8 e4m3 payload + bf16 scalars), byte layout:
      CT/ZT [V, 64 bf16]: 0:50 U/Ucen fp8, 50:101 g' fp8, 102:104 iv bf16,
                          104:106 c bf16
      NT    [V, 64 bf16]: 0:51 g' fp8, 52:54 iv bf16, 54:56 c bf16
    (fp8 on U/g' costs ~3e-4 relative loss error, far under the 2e-2 gate,
    and halves gather payload: 256B paired elements.)
  - Gathers use dma_gather (SWDGE). Its int16 index limit (<32768 rows) is
    handled by gathering PAIRED rows: index = id>>1 with elem_size = 2 rows,
    then one contiguous parity select (on f32-bitcast lanes) keeps the low
    bytes. 1024 idxs per instruction (65 of 128 SWDGE ring entries), spread
    over 4 queues.  SWDGE descriptor GENERATION (~5ns/desc x 4 Q7 queues)
    is the structural floor, so everything else is batched under it.
  - The center stream (8192 refs) is hoisted out of the block loop: gathered
    once up front, selected once, and its kl algebra runs once at the end.
    The hinge is also applied once at the end over all 16 blocks.
  - Each core processes 8192 batch rows in 16 gather-blocks of 512. Flat
    gather position i -> (partition i%128, slot i//128), so host index
    order is slot-major. Per gather block, batched over 4 sub-blocks:
      h = sum_j relu(U[ctx_j] + Ucen[cen]);  [h;1] @ [Wmu|Wls;bmu|bls] on PE
      A = exp(logsigma) + sum(mu^2)
      dc - dn = (g'_ctx - g'_neg) . h'   (fused difference dot)
  - Output per core: [128,2] partials; host reduces, applies -L/2, /B.
"""

import sys

for _p in ("/opt/trn_rl_repo", "/opt/pypackages"):
    if _p not in sys.path:
        sys.path.append(_p)

from contextlib import ExitStack

import numpy as np
import ml_dtypes

import concourse.bass as bass
import concourse.tile as tile
from concourse import bacc, mybir
from concourse.bass_utils import run_bass_kernel_spmd
from concourse.masks import make_identity

dt = mybir.dt
F32 = dt.float32
BF16 = dt.bfloat16
F8 = dt.float8e4
AF = mybir.ActivationFunctionType
OP = mybir.AluOpType
AX = mybir.AxisListType

V, D, H, L = 50000, 50, 50, 100
C = 10
B = 65536
NCORES = 8
NB = B // NCORES     # rows per core: 8192
GBS = 512            # rows per gather block
NGB = NB // GBS      # 16
NSB = GBS // 128     # 4 sub-blocks
Q = NSB * C          # 40 ctx slots per partition per gather block
NZ = NGB * NSB       # 64 cen slots per partition overall
E = 64               # table row width (bf16 elems, 128B)
MAXI = 1024          # idxs per dma_gather (65 of 128 SWDGE ring entries)
MARGIN = 1.0
# f32-lane select widths (payload bytes / 4, rounded up)
SELW = 27            # CT/ZT payload 106B
SELWN = 14           # NT payload 56B
IVC, CC = 51, 52     # bf16 col of iv/c in CT/ZT rows
IVN, CN = 26, 27     # bf16 col of iv/c in NT rows
W = 52               # dot lanes: H+1 padded to even (table pad byte is 0)

_CACHE: dict = {}


def _wrap_idx(flat):
    """int16 idx list -> [128, ceil(n/16)] wrapped-16, replicated across cores."""
    n = len(flat)
    nf = -(-n // 16)
    w = np.zeros((16, nf), np.int16)
    w[np.arange(n) % 16, np.arange(n) // 16] = flat
    return np.tile(w, (8, 1))


def _build_program():
    nc = bacc.Bacc("TRN2", target_bir_lowering=False, debug=False, num_swdge_queues=4)

    ct_d = nc.dram_tensor("ct", [V, E], BF16, kind="ExternalInput")
    nt_d = nc.dram_tensor("nt", [V, E], BF16, kind="ExternalInput")
    zt_d = nc.dram_tensor("zt", [V, E], BF16, kind="ExternalInput")
    wf_d = nc.dram_tensor("wf", [H + 1, L + 1], F32, kind="ExternalInput")
    # wrapped int16 half-indices: per gb [ctx 320 | neg 320 cols], then all
    # cen (512 cols) at the end
    IGC = Q * 128 // 16          # 320 idx cols per gb for ctx/neg streams
    IG = 2 * IGC
    IZ = NZ * 128 // 16          # 512 cen idx cols
    idx_d = nc.dram_tensor("idx", [128, NGB * IG + IZ], dt.int16,
                           kind="ExternalInput")
    # parity masks (uint8 0/1): per gb [ctx Q | neg Q], then cen NZ at the end
    MG = 2 * Q
    msk_d = nc.dram_tensor("msk", [128, NGB * MG + NZ], dt.uint8,
                           kind="ExternalInput")
    out_d = nc.dram_tensor("out", [128, 2], F32, kind="ExternalOutput")

    # paired views: half-row index k -> rows [2k, 2k+1] (256B elements)
    ct_v = bass.AP(ct_d, 0, [[2 * E, V // 2], [1, 2 * E]])
    nt_v = bass.AP(nt_d, 0, [[2 * E, V // 2], [1, 2 * E]])
    zt_v = bass.AP(zt_d, 0, [[2 * E, V // 2], [1, 2 * E]])

    def gather(out_ap, tab_v, idx_ap, n):
        nc.gpsimd.dma_gather(
            out_ap=out_ap, in_ap=tab_v, idxs_ap=idx_ap,
            num_idxs=n, num_idxs_reg=n, elem_size=2 * E, elem_step=2 * E,
            queue_num=0)

    # chunk a stream of `tot` idxs into ring-max gathers (1024 is a hard
    # ucode limit: 1920 crashes the device even though 121 ring entries fit)
    def gather_stream(tile_ap, tab_v, icols, tot):
        off = 0
        while off < tot:
            n = min(MAXI, tot - off)
            gather(tile_ap[:, off // 128:(off + n) // 128, :], tab_v,
                   icols[:, off // 16:(off + n) // 16], n)
            off += n

    with tile.TileContext(nc) as tc, ExitStack() as ctx:
        const = ctx.enter_context(tc.tile_pool(name="const", bufs=1))
        io = ctx.enter_context(tc.tile_pool(name="io", bufs=4))
        wk = ctx.enter_context(tc.tile_pool(name="wk", bufs=2))
        ps = ctx.enter_context(tc.tile_pool(name="ps", bufs=2, space="PSUM"))
        accp = ctx.enter_context(tc.tile_pool(name="accp", bufs=1))

        # warm up the SWDGE path (gpsimd lib load + per-queue ring init,
        # ~9us) during the input-DMA window: 4 tiny dummy gathers, one per
        # queue (idx 0 -> harmless reads of table row pair 0)
        warm_i = const.tile([128, 8], dt.int16)
        nc.vector.memset(warm_i[:], 0)
        for wq in range(4):
            warm_o = const.tile([128, 1, 2 * E], BF16)
            gather(warm_o[:], ct_v, warm_i[:], 128)

        # cen idxs first in their own tile so cen gathers start before the
        # big ctx/neg idx load lands; masks last (first select is late)
        idxz_sb = const.tile([128, IZ], dt.int16)
        nc.sync.dma_start(idxz_sb[:], idx_d.ap()[:, NGB * IG:])
        idx_sb = const.tile([128, NGB * IG], dt.int16)
        nc.sync.dma_start(idx_sb[:], idx_d.ap()[:, 0:NGB * IG])
        ident = const.tile([128, 128], F32)
        make_identity(nc, ident[:])
        wf_sb = const.tile([H + 1, L + 1], F32)
        nc.sync.dma_start(wf_sb[:], wf_d.ap())
        msk_sb = const.tile([128, NGB * MG + NZ], dt.uint8)
        nc.sync.dma_start(msk_sb[:], msk_d.ap())

        # --- center stream, hoisted: gather all NZ slots once ---
        CGall = accp.tile([128, NZ, 2 * E], BF16)
        gather_stream(CGall, zt_v, idxz_sb[:], NZ * 128)
        mz = msk_sb[:, NGB * MG:NGB * MG + NZ]
        nc.vector.copy_predicated(
            CGall[:, :, 0:2 * SELW].bitcast(F32),
            mz.unsqueeze(2).to_broadcast([128, NZ, SELW]),
            CGall[:, :, E:E + 2 * SELW].bitcast(F32))
        CGall8 = CGall[:].bitcast(F8)
        gzall = accp.tile([128, NZ, W], BF16)
        nc.scalar.copy(gzall[:], CGall8[:, :, D:D + W])

        # per-row stats, persisted across blocks for the batched end phase
        hball = accp.tile([128, NZ, W], BF16)
        Aall = accp.tile([128, NZ], F32)
        lsgall = accp.tile([128, NZ], F32)
        v1all = accp.tile([128, NGB, Q], F32)

        # end-phase state: hinge + cen kl, emitted in two chunks (blocks
        # 0..14 run while block 15's gathers drain; the rest in the tail)
        outt = accp.tile([128, 2], F32)
        hng = accp.tile([128, NGB, Q], F32)
        pzall = accp.tile([128, NZ, W], BF16)
        cdall = accp.tile([128, NZ], F32)
        caall = accp.tile([128, NZ], F32)
        ht = accp.tile([128, 4], F32)

        def end_phase(b0, b1, hcol, ccol):
            s0, s1 = b0 * NSB, b1 * NSB
            nc.scalar.activation(hng[:, b0:b1, :], v1all[:, b0:b1, :], AF.Relu,
                                 bias=float(MARGIN), scale=0.5)
            nc.vector.tensor_reduce(out=ht[:, hcol:hcol + 1],
                                    in_=hng[:, b0:b1, :], axis=AX.XY, op=OP.add)
            # cen kl: cw = (gz . h') + c + A*iv - lsg
            nc.vector.tensor_tensor(out=pzall[:, s0:s1, :], in0=gzall[:, s0:s1, :],
                                    in1=hball[:, s0:s1, :], op=OP.mult)
            nc.vector.tensor_reduce(out=cdall[:, s0:s1], in_=pzall[:, s0:s1, :],
                                    axis=AX.X, op=OP.add)
            nc.vector.tensor_tensor(out=caall[:, s0:s1], in0=CGall[:, s0:s1, IVC],
                                    in1=Aall[:, s0:s1], op=OP.mult)
            nc.vector.tensor_tensor(out=cdall[:, s0:s1], in0=cdall[:, s0:s1],
                                    in1=CGall[:, s0:s1, CC], op=OP.add)
            nc.vector.tensor_tensor(out=cdall[:, s0:s1], in0=cdall[:, s0:s1],
                                    in1=caall[:, s0:s1], op=OP.add)
            nc.vector.tensor_tensor(out=cdall[:, s0:s1], in0=cdall[:, s0:s1],
                                    in1=lsgall[:, s0:s1], op=OP.subtract)
            nc.vector.tensor_reduce(out=ht[:, ccol:ccol + 1], in_=cdall[:, s0:s1],
                                    axis=AX.X, op=OP.add)

        for gb in range(NGB):
            PG = io.tile([128, Q, 2 * E], BF16, tag="PG")     # ctx row pairs
            NG = io.tile([128, Q, 2 * E], BF16, tag="NG")     # neg row pairs
            s0 = gb * NSB

            icx = idx_sb[:, gb * IG:gb * IG + IGC]
            ing = idx_sb[:, gb * IG + IGC:(gb + 1) * IG]
            # ctx first (feeds the h pipeline); neg drains during the h phase
            gather_stream(PG, ct_v, icx, Q * 128)
            gather_stream(NG, nt_v, ing, Q * 128)

            # ctx parity select, in place, on f32-bitcast lanes
            mc = msk_sb[:, gb * MG:gb * MG + Q]
            mn = msk_sb[:, gb * MG + Q:(gb + 1) * MG]
            nc.vector.copy_predicated(
                PG[:, :, 0:2 * SELW].bitcast(F32),
                mc.unsqueeze(2).to_broadcast([128, Q, SELW]),
                PG[:, :, E:E + 2 * SELW].bitcast(F32))

            PG4 = PG[:].rearrange("p (s c) e -> p s c e", s=NSB)
            NG4 = NG[:].rearrange("p (s c) e -> p s c e", s=NSB)
            PG84 = PG[:].bitcast(F8).rearrange("p (s c) e -> p s c e", s=NSB)
            NG84 = NG[:].bitcast(F8).rearrange("p (s c) e -> p s c e", s=NSB)

            # h = sum_j relu(U_ctx + U_cen), one batched pass per block
            y4 = wk.tile([128, NSB, C, D], BF16, tag="y4")
            nc.vector.tensor_tensor(
                out=y4[:], in0=PG84[:, :, :, 0:D],
                in1=CGall8[:, s0:s0 + NSB, 0:D].unsqueeze(2)
                    .to_broadcast([128, NSB, C, D]),
                op=OP.add)
            r4 = wk.tile([128, NSB, C, D], BF16, tag="r4")
            nc.scalar.activation(r4[:], y4[:], AF.Relu)
            # h = sum_j r via contiguous folds (strided X-reduce is slow)
            f1 = wk.tile([128, NSB, 5, D], BF16, tag="f1")
            nc.vector.tensor_tensor(out=f1[:], in0=r4[:, :, 0:5, :],
                                    in1=r4[:, :, 5:10, :], op=OP.add)
            f2 = wk.tile([128, NSB, 2, D], BF16, tag="f2")
            nc.vector.tensor_tensor(out=f2[:], in0=f1[:, :, 0:2, :],
                                    in1=f1[:, :, 2:4, :], op=OP.add)
            h4 = wk.tile([128, NSB, W], F32, tag="h4")
            nc.vector.tensor_tensor(out=h4[:, :, 0:D], in0=f2[:, :, 0, :],
                                    in1=f2[:, :, 1, :], op=OP.add)
            nc.vector.tensor_tensor(out=h4[:, :, 0:D], in0=h4[:, :, 0:D],
                                    in1=f1[:, :, 4, :], op=OP.add)
            nc.vector.memset(h4[:, :, H:H + 1], 1.0)
            nc.vector.memset(h4[:, :, H + 1:W], 0.0)
            nc.scalar.copy(hball[:, s0:s0 + NSB, :], h4[:])

            # mu = h' @ wf on PE, per sub-block; batched epilogue
            hT_ps = ps.tile([64, NSB * 128], F32, tag="hTp")
            for s in range(NSB):
                nc.tensor.transpose(hT_ps[0:H + 1, s * 128:(s + 1) * 128],
                                    h4[:, s, 0:H + 1], ident[:])
            hT = wk.tile([64, NSB * 128], F32, tag="hT")
            nc.scalar.copy(hT[0:H + 1, :], hT_ps[0:H + 1, :])
            mu_ps = ps.tile([128, NSB, L + 1], F32, tag="mu")
            for s in range(NSB):
                nc.tensor.matmul(mu_ps[:, s, :],
                                 lhsT=hT[0:H + 1, s * 128:(s + 1) * 128],
                                 rhs=wf_sb[:], start=True, stop=True)
            sqj = wk.tile([128, L], F32, tag="sqj")
            for s in range(NSB):
                nc.scalar.activation(sqj[:], mu_ps[:, s, 0:L], AF.Square,
                                     accum_out=Aall[:, s0 + s:s0 + s + 1])
            sig = wk.tile([128, NSB], F32, tag="sig")
            nc.scalar.activation(sig[:], mu_ps[:, :, L], AF.Exp)
            nc.scalar.copy(lsgall[:, s0:s0 + NSB], mu_ps[:, :, L])

            # neg select (no dep on the scalar chain), then the fused
            # ctx-neg difference dot: dc - dn = ((gc - gn) . h'), then
            # d = (dc-dn) + (cc-cn) + A*(ivc-ivn) with the hinge deferred.
            # The last block runs this in two sub-block halves so the first
            # half only waits on the first 3 of its 5 neg gather chunks,
            # shortening the post-last-gather tail.
            def dot_phase(a, b, add_A):
                n = b - a
                sl = slice(a * C, b * C)
                nc.vector.copy_predicated(
                    NG[:, sl, 0:2 * SELWN].bitcast(F32),
                    mn[:, sl].unsqueeze(2).to_broadcast([128, n * C, SELWN]),
                    NG[:, sl, E:E + 2 * SELWN].bitcast(F32))
                gd = wk.tile([128, n, C, W], BF16, tag=f"gd{a}{b}")
                nc.vector.tensor_tensor(out=gd[:], in0=PG84[:, a:b, :, D:D + W],
                                        in1=NG84[:, a:b, :, 0:W], op=OP.subtract)
                pd = wk.tile([128, n, C, W], BF16, tag=f"pd{a}{b}")
                nc.vector.tensor_tensor(
                    out=pd[:], in0=gd[:],
                    in1=hball[:, s0 + a:s0 + b, :].unsqueeze(2)
                        .to_broadcast([128, n, C, W]),
                    op=OP.mult)
                v1 = v1all[:, gb, sl].rearrange("p (s c) -> p s c", s=n)
                nc.vector.tensor_reduce(out=v1, in_=pd[:], axis=AX.X, op=OP.add)
                v2 = wk.tile([128, n, C], F32, tag=f"v2{a}{b}")
                nc.vector.tensor_tensor(out=v2[:], in0=PG4[:, a:b, :, CC],
                                        in1=NG4[:, a:b, :, CN], op=OP.subtract)
                v3 = wk.tile([128, n, C], F32, tag=f"v3{a}{b}")
                nc.vector.tensor_tensor(out=v3[:], in0=PG4[:, a:b, :, IVC],
                                        in1=NG4[:, a:b, :, IVN], op=OP.subtract)
                if add_A:
                    nc.vector.tensor_tensor(out=Aall[:, s0:s0 + NSB],
                                            in0=Aall[:, s0:s0 + NSB], in1=sig[:],
                                            op=OP.add)
                nc.vector.tensor_tensor(
                    out=v3[:], in0=v3[:],
                    in1=Aall[:, s0 + a:s0 + b].unsqueeze(2)
                        .to_broadcast([128, n, C]), op=OP.mult)
                nc.vector.tensor_tensor(out=v1, in0=v1, in1=v2[:], op=OP.add)
                nc.vector.tensor_tensor(out=v1, in0=v1, in1=v3[:], op=OP.add)

            if gb == NGB - 1:
                dot_phase(0, 2, True)
                dot_phase(2, NSB, False)
            else:
                dot_phase(0, NSB, True)

            if gb == NGB - 3:
                end_phase(0, NGB - 2, 0, 2)

        end_phase(NGB - 2, NGB, 1, 3)
        nc.vector.tensor_tensor(out=outt[:, 0:1], in0=ht[:, 0:1], in1=ht[:, 1:2],
                                op=OP.add)
        nc.vector.tensor_tensor(out=outt[:, 1:2], in0=ht[:, 2:3], in1=ht[:, 3:4],
                                op=OP.add)
        nc.sync.dma_start(out_d.ap(), outt[:])

    # Spread gathers across the 4 SWDGE queues (4 Q7 core-pairs run desc-gen
    # in parallel). queue = Tile-assigned DMASW sem lane % 4 keeps per-lane
    # completion FIFO within its queue, so Tile's sem ordering stays sound.
    import re
    for inst in nc.inst_map.values():
        if type(inst).__name__ == "InstDMAGatherAnt" and inst.sync_info:
            for u in inst.sync_info.on_update:
                m = re.match(r"DMASW(\d+)_", u.ant_name or "")
                if m:
                    inst.queue_num = int(m.group(1)) % 4
                    break
    nc.compile()
    return nc


def _prep_inputs(emb, W1, b1, Wmu, bmu, Wls, bls, type_means_tbl,
                 type_logvars_tbl, centers, contexts, neg_contexts):
    emb = np.asarray(emb, np.float32)
    W1 = np.asarray(W1, np.float32)
    U = emb @ W1[:D]
    Ucen = emb @ W1[D:] + np.asarray(b1, np.float32)

    tm = np.asarray(type_means_tbl, np.float32)
    lv = np.asarray(type_logvars_tbl, np.float32)[:, 0]
    sq = (tm * tm).sum(axis=1)
    iv = np.exp(-lv)

    wf = np.zeros((H + 1, L + 1), np.float32)
    wf[0:H, 0:L] = np.asarray(Wmu, np.float32)
    wf[0:H, L] = np.asarray(Wls, np.float32)[:, 0]
    wf[H, 0:L] = np.asarray(bmu, np.float32)
    wf[H, L] = np.asarray(bls, np.float32)[0]

    G = (tm @ wf[0:H + 1, 0:L].T) * (-2.0 * iv)[:, None]    # [V, H+1]
    c = sq * iv + lv

    u8 = lambda x: x.astype(ml_dtypes.float8_e4m3).view(np.uint8)
    b8 = lambda x: x.astype(ml_dtypes.bfloat16).view(np.uint8)
    ctb = np.zeros((V, 2 * E), np.uint8)
    ctb[:, 0:D] = u8(U)
    ctb[:, D:D + H + 1] = u8(G)
    ctb[:, 2 * IVC:2 * IVC + 2] = b8(iv).reshape(V, 2)
    ctb[:, 2 * CC:2 * CC + 2] = b8(c).reshape(V, 2)
    ztb = ctb.copy()
    ztb[:, 0:D] = u8(Ucen)
    ntb = np.zeros((V, 2 * E), np.uint8)
    ntb[:, 0:H + 1] = u8(G)
    ntb[:, 2 * IVN:2 * IVN + 2] = b8(iv).reshape(V, 2)
    ntb[:, 2 * CN:2 * CN + 2] = b8(c).reshape(V, 2)
    ct = ctb.view(ml_dtypes.bfloat16)
    zt = ztb.view(ml_dtypes.bfloat16)
    nt = ntb.view(ml_dtypes.bfloat16)

    # flat gather order: position i = slot*128 + p; slot = s*C + j for ctx/neg
    # (per block), slot = gb*NSB + s for cen; b = core*NB + gb*GBS + s*128 + p
    cx = np.asarray(contexts, np.int32).reshape(NCORES, NGB, NSB, 128, C)
    ng = np.asarray(neg_contexts, np.int32).reshape(NCORES, NGB, NSB, 128, C)
    cn = np.asarray(centers, np.int32).reshape(NCORES, NGB, NSB, 128)
    # -> [core, gb, slot(s,j), p] flat per stream
    cxf = cx.transpose(0, 1, 2, 4, 3).reshape(NCORES, NGB, Q * 128)
    ngf = ng.transpose(0, 1, 2, 4, 3).reshape(NCORES, NGB, Q * 128)
    cnf = cn.reshape(NCORES, NZ * 128)

    in_maps = []
    for cix in range(NCORES):
        iparts, mparts = [], []
        for gb in range(NGB):
            for f in (cxf[cix, gb], ngf[cix, gb]):
                iparts.append(_wrap_idx((f >> 1).astype(np.int16)))
            # masks in [p, slot] layout
            mparts.append(np.ascontiguousarray(
                (cxf[cix, gb] & 1).reshape(Q, 128).T.astype(np.uint8)))
            mparts.append(np.ascontiguousarray(
                (ngf[cix, gb] & 1).reshape(Q, 128).T.astype(np.uint8)))
        iparts.append(_wrap_idx((cnf[cix] >> 1).astype(np.int16)))
        mparts.append(np.ascontiguousarray(
            (cnf[cix] & 1).reshape(NZ, 128).T.astype(np.uint8)))
        in_maps.append({
            "ct": ct, "nt": nt, "zt": zt, "wf": wf,
            "idx": np.concatenate(iparts, axis=1),
            "msk": np.concatenate(mparts, axis=1),
        })
    return in_maps


def kernel(**inputs) -> np.ndarray:
    if "nc" not in _CACHE:
        _CACHE["nc"] = _build_program()
    nc = _CACHE["nc"]
    in_maps = _prep_inputs(**inputs)
    res = run_bass_kernel_spmd(nc, in_maps, core_ids=list(range(NCORES)))
    total = 0.0
    for cix in range(NCORES):
        out = np.asarray(res.results[cix]["out"], np.float64)
        total += out[:, 0].sum() + 0.5 * out[:, 1].sum()
    loss = total / B - L / 2.0
    return np.float32(loss)


# revision 30
# speedup vs baseline: 1.0258x; 1.0258x over previous
"""Trainium2 Bass kernel for the BSG word2gauss-style hinge/KL loss.

Strategy (data-parallel over 8 NeuronCores):
  - Host precomputes gather tables (batch-independent weight prep).
    Key algebra: 2*kl + L = A_b*iv_w + h'_b . g'_w + c_w - lsg_b with
      A_b  = exp(lsg_b) + sum(mu_b^2)
      g'_w = -2*iv_w*(wf[:, :L] @ tm_w)  in R^{H+1}   (projected type mean)
      c_w  = sq_w*iv_w + lv_w
    so the per-(row, word) interaction is a 51-dim dot with h' = [h; 1]
    instead of a 100-dim dot with mu, and no mu2 scaling pass is needed.
    Tables, 128B rows (fp8 e4m3 payload + bf16 scalars), byte layout:
      CT/ZT [V, 64 bf16]: 0:50 U/Ucen fp8, 50:101 g' fp8, 102:104 iv bf16,
                          104:106 c bf16
      NT    [V, 64 bf16]: 0:51 g' fp8, 52:54 iv bf16, 54:56 c bf16
    (fp8 on U/g' costs ~3e-4 relative loss error, far under the 2e-2 gate,
    and halves gather payload: 256B paired elements.)
  - Gathers use dma_gather (SWDGE). Its int16 index limit (<32768 rows) is
    handled by gathering PAIRED rows: index = id>>1 with elem_size = 2 rows,
    then one contiguous parity select (on f32-bitcast lanes) keeps the low
    bytes. 1024 idxs per instruction (65 of 128 SWDGE ring entries), spread
    over 4 queues.  SWDGE descriptor GENERATION (~5ns/desc x 4 Q7 queues)
    is the structural floor, so everything else is batched under it.
  - The center stream (8192 refs) is hoisted out of the block loop: gathered
    once up front, selected once, and its kl algebra runs once at the end.
    The hinge is also applied once at the end over all 16 blocks.
  - Each core processes 8192 batch rows in 16 gather-blocks of 512. Flat
    gather position i -> (partition i%128, slot i//128), so host index
    order is slot-major. Per gather block, batched over 4 sub-blocks:
      h = sum_j relu(U[ctx_j] + Ucen[cen]);  [h;1] @ [Wmu|Wls;bmu|bls] on PE
      A = exp(logsigma) + sum(mu^2)
      dc - dn = (g'_ctx - g'_neg) . h'   (fused difference dot)
  - Output per core: [128,2] partials; host reduces, applies -L/2, /B.
"""

import sys

for _p in ("/opt/trn_rl_repo", "/opt/pypackages"):
    if _p not in sys.path:
        sys.path.append(_p)

from contextlib import ExitStack

import numpy as np
import ml_dtypes

import concourse.bass as bass
import concourse.tile as tile
from concourse import bacc, mybir
from concourse.bass_utils import run_bass_kernel_spmd
from concourse.masks import make_identity

dt = mybir.dt
F32 = dt.float32
BF16 = dt.bfloat16
F8 = dt.float8e4
AF = mybir.ActivationFunctionType
OP = mybir.AluOpType
AX = mybir.AxisListType

V, D, H, L = 50000, 50, 50, 100
C = 10
B = 65536
NCORES = 8
NB = B // NCORES     # rows per core: 8192
GBS = 512            # rows per gather block
NGB = NB // GBS      # 16
NSB = GBS // 128     # 4 sub-blocks
Q = NSB * C          # 40 ctx slots per partition per gather block
NZ = NGB * NSB       # 64 cen slots per partition overall
E = 64               # table row width (bf16 elems, 128B)
MAXI = 1024          # idxs per dma_gather (65 of 128 SWDGE ring entries)
MARGIN = 1.0
# f32-lane select widths (payload bytes / 4, rounded up)
SELW = 27            # CT/ZT payload 106B
SELWN = 14           # NT payload 56B
IVC, CC = 51, 52     # bf16 col of iv/c in CT/ZT rows
IVN, CN = 26, 27     # bf16 col of iv/c in NT rows
W = 52               # dot lanes: H+1 padded to even (table pad byte is 0)

_CACHE: dict = {}


def _wrap_idx(flat):
    """int16 idx list -> [128, ceil(n/16)] wrapped-16, replicated across cores."""
    n = len(flat)
    nf = -(-n // 16)
    w = np.zeros((16, nf), np.int16)
    w[np.arange(n) % 16, np.arange(n) // 16] = flat
    return np.tile(w, (8, 1))


def _build_program():
    nc = bacc.Bacc("TRN2", target_bir_lowering=False, debug=False, num_swdge_queues=4)

    ct_d = nc.dram_tensor("ct", [V, E], BF16, kind="ExternalInput")
    nt_d = nc.dram_tensor("nt", [V, E], BF16, kind="ExternalInput")
    zt_d = nc.dram_tensor("zt", [V, E], BF16, kind="ExternalInput")
    wf_d = nc.dram_tensor("wf", [H + 1, L + 1], F32, kind="ExternalInput")
    # wrapped int16 half-indices: per gb [ctx 320 | neg 320 cols], then all
    # cen (512 cols) at the end
    IGC = Q * 128 // 16          # 320 idx cols per gb for ctx/neg streams
    IG = 2 * IGC
    IZ = NZ * 128 // 16          # 512 cen idx cols
    idx_d = nc.dram_tensor("idx", [128, NGB * IG + IZ], dt.int16,
                           kind="ExternalInput")
    # parity masks (uint8 0/1): per gb [ctx Q | neg Q], then cen NZ at the end
    MG = 2 * Q
    msk_d = nc.dram_tensor("msk", [128, NGB * MG + NZ], dt.uint8,
                           kind="ExternalInput")
    out_d = nc.dram_tensor("out", [128, 2], F32, kind="ExternalOutput")

    # paired views: half-row index k -> rows [2k, 2k+1] (256B elements)
    ct_v = bass.AP(ct_d, 0, [[2 * E, V // 2], [1, 2 * E]])
    nt_v = bass.AP(nt_d, 0, [[2 * E, V // 2], [1, 2 * E]])
    zt_v = bass.AP(zt_d, 0, [[2 * E, V // 2], [1, 2 * E]])

    def gather(out_ap, tab_v, idx_ap, n):
        nc.gpsimd.dma_gather(
            out_ap=out_ap, in_ap=tab_v, idxs_ap=idx_ap,
            num_idxs=n, num_idxs_reg=n, elem_size=2 * E, elem_step=2 * E,
            queue_num=0)

    # chunk a stream of `tot` idxs into ring-max gathers (1024 is a hard
    # ucode limit: 1920 crashes the device even though 121 ring entries fit)
    def gather_stream(tile_ap, tab_v, icols, tot):
        off = 0
        while off < tot:
            n = min(MAXI, tot - off)
            gather(tile_ap[:, off // 128:(off + n) // 128, :], tab_v,
                   icols[:, off // 16:(off + n) // 16], n)
            off += n

    with tile.TileContext(nc) as tc, ExitStack() as ctx:
        const = ctx.enter_context(tc.tile_pool(name="const", bufs=1))
        io = ctx.enter_context(tc.tile_pool(name="io", bufs=4))
        wk = ctx.enter_context(tc.tile_pool(name="wk", bufs=2))
        ps = ctx.enter_context(tc.tile_pool(name="ps", bufs=2, space="PSUM"))
        accp = ctx.enter_context(tc.tile_pool(name="accp", bufs=1))

        # cen idxs first in their own tile so cen gathers start before the
        # big ctx/neg idx load lands; masks last (first select is late)
        # (note: warmup dummy gathers were tried and hurt -- the ~18us Pool
        # library-load gates any gather, so dummies only serialize in front)
        idxz_sb = const.tile([128, IZ], dt.int16)
        nc.sync.dma_start(idxz_sb[:], idx_d.ap()[:, NGB * IG:])
        idx_sb = const.tile([128, NGB * IG], dt.int16)
        nc.sync.dma_start(idx_sb[:], idx_d.ap()[:, 0:NGB * IG])
        ident = const.tile([128, 128], F32)
        make_identity(nc, ident[:])
        wf_sb = const.tile([H + 1, L + 1], F32)
        nc.sync.dma_start(wf_sb[:], wf_d.ap())
        msk_sb = const.tile([128, NGB * MG + NZ], dt.uint8)
        nc.sync.dma_start(msk_sb[:], msk_d.ap())

        # --- center stream, hoisted: gather all NZ slots once ---
        CGall = accp.tile([128, NZ, 2 * E], BF16)
        gather_stream(CGall, zt_v, idxz_sb[:], NZ * 128)
        mz = msk_sb[:, NGB * MG:NGB * MG + NZ]
        nc.vector.copy_predicated(
            CGall[:, :, 0:2 * SELW].bitcast(F32),
            mz.unsqueeze(2).to_broadcast([128, NZ, SELW]),
            CGall[:, :, E:E + 2 * SELW].bitcast(F32))
        CGall8 = CGall[:].bitcast(F8)
        gzall = accp.tile([128, NZ, W], BF16)
        nc.scalar.copy(gzall[:], CGall8[:, :, D:D + W])

        # per-row stats, persisted across blocks for the batched end phase
        hball = accp.tile([128, NZ, W], BF16)
        Aall = accp.tile([128, NZ], F32)
        lsgall = accp.tile([128, NZ], F32)
        v1all = accp.tile([128, NGB, Q], F32)

        # end-phase state: hinge + cen kl, emitted in two chunks (blocks
        # 0..14 run while block 15's gathers drain; the rest in the tail)
        outt = accp.tile([128, 2], F32)
        hng = accp.tile([128, NGB, Q], F32)
        pzall = accp.tile([128, NZ, W], BF16)
        cdall = accp.tile([128, NZ], F32)
        caall = accp.tile([128, NZ], F32)
        ht = accp.tile([128, 4], F32)

        def end_phase(b0, b1, hcol, ccol):
            s0, s1 = b0 * NSB, b1 * NSB
            nc.scalar.activation(hng[:, b0:b1, :], v1all[:, b0:b1, :], AF.Relu,
                                 bias=float(MARGIN), scale=0.5)
            nc.vector.tensor_reduce(out=ht[:, hcol:hcol + 1],
                                    in_=hng[:, b0:b1, :], axis=AX.XY, op=OP.add)
            # cen kl: cw = (gz . h') + c + A*iv - lsg
            nc.vector.tensor_tensor(out=pzall[:, s0:s1, :], in0=gzall[:, s0:s1, :],
                                    in1=hball[:, s0:s1, :], op=OP.mult)
            nc.vector.tensor_reduce(out=cdall[:, s0:s1], in_=pzall[:, s0:s1, :],
                                    axis=AX.X, op=OP.add)
            nc.vector.tensor_tensor(out=caall[:, s0:s1], in0=CGall[:, s0:s1, IVC],
                                    in1=Aall[:, s0:s1], op=OP.mult)
            nc.vector.tensor_tensor(out=cdall[:, s0:s1], in0=cdall[:, s0:s1],
                                    in1=CGall[:, s0:s1, CC], op=OP.add)
            nc.vector.tensor_tensor(out=cdall[:, s0:s1], in0=cdall[:, s0:s1],
                                    in1=caall[:, s0:s1], op=OP.add)
            nc.vector.tensor_tensor(out=cdall[:, s0:s1], in0=cdall[:, s0:s1],
                                    in1=lsgall[:, s0:s1], op=OP.subtract)
            nc.vector.tensor_reduce(out=ht[:, ccol:ccol + 1], in_=cdall[:, s0:s1],
                                    axis=AX.X, op=OP.add)

        for gb in range(NGB):
            PG = io.tile([128, Q, 2 * E], BF16, tag="PG")     # ctx row pairs
            NG = io.tile([128, Q, 2 * E], BF16, tag="NG")     # neg row pairs
            s0 = gb * NSB

            icx = idx_sb[:, gb * IG:gb * IG + IGC]
            ing = idx_sb[:, gb * IG + IGC:(gb + 1) * IG]
            # ctx first (feeds the h pipeline); neg drains during the h phase
            gather_stream(PG, ct_v, icx, Q * 128)
            gather_stream(NG, nt_v, ing, Q * 128)

            # ctx parity select, in place, on f32-bitcast lanes
            mc = msk_sb[:, gb * MG:gb * MG + Q]
            mn = msk_sb[:, gb * MG + Q:(gb + 1) * MG]
            nc.vector.copy_predicated(
                PG[:, :, 0:2 * SELW].bitcast(F32),
                mc.unsqueeze(2).to_broadcast([128, Q, SELW]),
                PG[:, :, E:E + 2 * SELW].bitcast(F32))

            PG4 = PG[:].rearrange("p (s c) e -> p s c e", s=NSB)
            NG4 = NG[:].rearrange("p (s c) e -> p s c e", s=NSB)
            PG84 = PG[:].bitcast(F8).rearrange("p (s c) e -> p s c e", s=NSB)
            NG84 = NG[:].bitcast(F8).rearrange("p (s c) e -> p s c e", s=NSB)

            # h = sum_j relu(U_ctx + U_cen), one batched pass per block
            y4 = wk.tile([128, NSB, C, D], BF16, tag="y4")
            nc.vector.tensor_tensor(
                out=y4[:], in0=PG84[:, :, :, 0:D],
                in1=CGall8[:, s0:s0 + NSB, 0:D].unsqueeze(2)
                    .to_broadcast([128, NSB, C, D]),
                op=OP.add)
            r4 = wk.tile([128, NSB, C, D], BF16, tag="r4")
            nc.scalar.activation(r4[:], y4[:], AF.Relu)
            # h = sum_j r via contiguous folds (strided X-reduce is slow)
            f1 = wk.tile([128, NSB, 5, D], BF16, tag="f1")
            nc.vector.tensor_tensor(out=f1[:], in0=r4[:, :, 0:5, :],
                                    in1=r4[:, :, 5:10, :], op=OP.add)
            f2 = wk.tile([128, NSB, 2, D], BF16, tag="f2")
            nc.vector.tensor_tensor(out=f2[:], in0=f1[:, :, 0:2, :],
                                    in1=f1[:, :, 2:4, :], op=OP.add)
            h4 = wk.tile([128, NSB, W], F32, tag="h4")
            nc.vector.tensor_tensor(out=h4[:, :, 0:D], in0=f2[:, :, 0, :],
                                    in1=f2[:, :, 1, :], op=OP.add)
            nc.vector.tensor_tensor(out=h4[:, :, 0:D], in0=h4[:, :, 0:D],
                                    in1=f1[:, :, 4, :], op=OP.add)
            nc.vector.memset(h4[:, :, H:H + 1], 1.0)
            nc.vector.memset(h4[:, :, H + 1:W], 0.0)
            nc.scalar.copy(hball[:, s0:s0 + NSB, :], h4[:])

            # mu = h' @ wf on PE, per sub-block; batched epilogue
            hT_ps = ps.tile([64, NSB * 128], F32, tag="hTp")
            for s in range(NSB):
                nc.tensor.transpose(hT_ps[0:H + 1, s * 128:(s + 1) * 128],
                                    h4[:, s, 0:H + 1], ident[:])
            hT = wk.tile([64, NSB * 128], F32, tag="hT")
            nc.scalar.copy(hT[0:H + 1, :], hT_ps[0:H + 1, :])
            mu_ps = ps.tile([128, NSB, L + 1], F32, tag="mu")
            for s in range(NSB):
                nc.tensor.matmul(mu_ps[:, s, :],
                                 lhsT=hT[0:H + 1, s * 128:(s + 1) * 128],
                                 rhs=wf_sb[:], start=True, stop=True)
            sqj = wk.tile([128, L], F32, tag="sqj")
            for s in range(NSB):
                nc.scalar.activation(sqj[:], mu_ps[:, s, 0:L], AF.Square,
                                     accum_out=Aall[:, s0 + s:s0 + s + 1])
            sig = wk.tile([128, NSB], F32, tag="sig")
            nc.scalar.activation(sig[:], mu_ps[:, :, L], AF.Exp)
            nc.scalar.copy(lsgall[:, s0:s0 + NSB], mu_ps[:, :, L])

            # neg select (no dep on the scalar chain), then the fused
            # ctx-neg difference dot: dc - dn = ((gc - gn) . h'), then
            # d = (dc-dn) + (cc-cn) + A*(ivc-ivn) with the hinge deferred.
            # The last block runs this in two sub-block halves so the first
            # half only waits on the first 3 of its 5 neg gather chunks,
            # shortening the post-last-gather tail.
            def dot_phase(a, b, add_A):
                n = b - a
                sl = slice(a * C, b * C)
                nc.vector.copy_predicated(
                    NG[:, sl, 0:2 * SELWN].bitcast(F32),
                    mn[:, sl].unsqueeze(2).to_broadcast([128, n * C, SELWN]),
                    NG[:, sl, E:E + 2 * SELWN].bitcast(F32))
                gd = wk.tile([128, n, C, W], BF16, tag=f"gd{a}{b}")
                nc.vector.tensor_tensor(out=gd[:], in0=PG84[:, a:b, :, D:D + W],
                                        in1=NG84[:, a:b, :, 0:W], op=OP.subtract)
                pd = wk.tile([128, n, C, W], BF16, tag=f"pd{a}{b}")
                nc.vector.tensor_tensor(
                    out=pd[:], in0=gd[:],
                    in1=hball[:, s0 + a:s0 + b, :].unsqueeze(2)
                        .to_broadcast([128, n, C, W]),
                    op=OP.mult)
                v1 = v1all[:, gb, sl].rearrange("p (s c) -> p s c", s=n)
                nc.vector.tensor_reduce(out=v1, in_=pd[:], axis=AX.X, op=OP.add)
                v2 = wk.tile([128, n, C], F32, tag=f"v2{a}{b}")
                nc.vector.tensor_tensor(out=v2[:], in0=PG4[:, a:b, :, CC],
                                        in1=NG4[:, a:b, :, CN], op=OP.subtract)
                v3 = wk.tile([128, n, C], F32, tag=f"v3{a}{b}")
                nc.vector.tensor_tensor(out=v3[:], in0=PG4[:, a:b, :, IVC],
                                        in1=NG4[:, a:b, :, IVN], op=OP.subtract)
                if add_A:
                    nc.vector.tensor_tensor(out=Aall[:, s0:s0 + NSB],
                                            in0=Aall[:, s0:s0 + NSB], in1=sig[:],
                                            op=OP.add)
                nc.vector.tensor_tensor(
                    out=v3[:], in0=v3[:],
                    in1=Aall[:, s0 + a:s0 + b].unsqueeze(2)
                        .to_broadcast([128, n, C]), op=OP.mult)
                nc.vector.tensor_tensor(out=v1, in0=v1, in1=v2[:], op=OP.add)
                nc.vector.tensor_tensor(out=v1, in0=v1, in1=v3[:], op=OP.add)

            dot_phase(0, NSB, True)

            if gb == NGB - 3:
                end_phase(0, NGB - 2, 0, 2)

        end_phase(NGB - 2, NGB, 1, 3)
        nc.vector.tensor_tensor(out=outt[:, 0:1], in0=ht[:, 0:1], in1=ht[:, 1:2],
                                op=OP.add)
        nc.vector.tensor_tensor(out=outt[:, 1:2], in0=ht[:, 2:3], in1=ht[:, 3:4],
                                op=OP.add)
        nc.sync.dma_start(out_d.ap(), outt[:])

    # Spread gathers across the 4 SWDGE queues (4 Q7 core-pairs run desc-gen
    # in parallel). queue = Tile-assigned DMASW sem lane % 4 keeps per-lane
    # completion FIFO within its queue, so Tile's sem ordering stays sound.
    import re
    for inst in nc.inst_map.values():
        if type(inst).__name__ == "InstDMAGatherAnt" and inst.sync_info:
            for u in inst.sync_info.on_update:
                m = re.match(r"DMASW(\d+)_", u.ant_name or "")
                if m:
                    inst.queue_num = int(m.group(1)) % 4
                    break
    nc.compile()
    return nc


def _prep_inputs(emb, W1, b1, Wmu, bmu, Wls, bls, type_means_tbl,
                 type_logvars_tbl, centers, contexts, neg_contexts):
    emb = np.asarray(emb, np.float32)
    W1 = np.asarray(W1, np.float32)
    U = emb @ W1[:D]
    Ucen = emb @ W1[D:] + np.asarray(b1, np.float32)

    tm = np.asarray(type_means_tbl, np.float32)
    lv = np.asarray(type_logvars_tbl, np.float32)[:, 0]
    sq = (tm * tm).sum(axis=1)
    iv = np.exp(-lv)

    wf = np.zeros((H + 1, L + 1), np.float32)
    wf[0:H, 0:L] = np.asarray(Wmu, np.float32)
    wf[0:H, L] = np.asarray(Wls, np.float32)[:, 0]
    wf[H, 0:L] = np.asarray(bmu, np.float32)
    wf[H, L] = np.asarray(bls, np.float32)[0]

    G = (tm @ wf[0:H + 1, 0:L].T) * (-2.0 * iv)[:, None]    # [V, H+1]
    c = sq * iv + lv

    u8 = lambda x: x.astype(ml_dtypes.float8_e4m3).view(np.uint8)
    b8 = lambda x: x.astype(ml_dtypes.bfloat16).view(np.uint8)
    ctb = np.zeros((V, 2 * E), np.uint8)
    ctb[:, 0:D] = u8(U)
    ctb[:, D:D + H + 1] = u8(G)
    ctb[:, 2 * IVC:2 * IVC + 2] = b8(iv).reshape(V, 2)
    ctb[:, 2 * CC:2 * CC + 2] = b8(c).reshape(V, 2)
    ztb = ctb.copy()
    ztb[:, 0:D] = u8(Ucen)
    ntb = np.zeros((V, 2 * E), np.uint8)
    ntb[:, 0:H + 1] = u8(G)
    ntb[:, 2 * IVN:2 * IVN + 2] = b8(iv).reshape(V, 2)
    ntb[:, 2 * CN:2 * CN + 2] = b8(c).reshape(V, 2)
    ct = ctb.view(ml_dtypes.bfloat16)
    zt = ztb.view(ml_dtypes.bfloat16)
    nt = ntb.view(ml_dtypes.bfloat16)

    # flat gather order: position i = slot*128 + p; slot = s*C + j for ctx/neg
    # (per block), slot = gb*NSB + s for cen; b = core*NB + gb*GBS + s*128 + p
    cx = np.asarray(contexts, np.int32).reshape(NCORES, NGB, NSB, 128, C)
    ng = np.asarray(neg_contexts, np.int32).reshape(NCORES, NGB, NSB, 128, C)
    cn = np.asarray(centers, np.int32).reshape(NCORES, NGB, NSB, 128)
    # -> [core, gb, slot(s,j), p] flat per stream
    cxf = cx.transpose(0, 1, 2, 4, 3).reshape(NCORES, NGB, Q * 128)
    ngf = ng.transpose(0, 1, 2, 4, 3).reshape(NCORES, NGB, Q * 128)
    cnf = cn.reshape(NCORES, NZ * 128)

    in_maps = []
    for cix in range(NCORES):
        iparts, mparts = [], []
        for gb in range(NGB):
            for f in (cxf[cix, gb], ngf[cix, gb]):
                iparts.append(_wrap_idx((f >> 1).astype(np.int16)))
            # masks in [p, slot] layout
            mparts.append(np.ascontiguousarray(
                (cxf[cix, gb] & 1).reshape(Q, 128).T.astype(np.uint8)))
            mparts.append(np.ascontiguousarray(
                (ngf[cix, gb] & 1).reshape(Q, 128).T.astype(np.uint8)))
        iparts.append(_wrap_idx((cnf[cix] >> 1).astype(np.int16)))
        mparts.append(np.ascontiguousarray(
            (cnf[cix] & 1).reshape(NZ, 128).T.astype(np.uint8)))
        in_maps.append({
            "ct": ct, "nt": nt, "zt": zt, "wf": wf,
            "idx": np.concatenate(iparts, axis=1),
            "msk": np.concatenate(mparts, axis=1),
        })
    return in_maps


def kernel(**inputs) -> np.ndarray:
    if "nc" not in _CACHE:
        _CACHE["nc"] = _build_program()
    nc = _CACHE["nc"]
    in_maps = _prep_inputs(**inputs)
    res = run_bass_kernel_spmd(nc, in_maps, core_ids=list(range(NCORES)))
    total = 0.0
    for cix in range(NCORES):
        out = np.asarray(res.results[cix]["out"], np.float64)
        total += out[:, 0].sum() + 0.5 * out[:, 1].sum()
    loss = total / B - L / 2.0
    return np.float32(loss)
